# revision 1
# baseline (speedup 1.0000x reference)
"""Causal multi-head attention (B=4, S=1024, D=1024, H=16, hd=64) on 8 TRN2 cores.

Sharding: head-parallel. Core c owns heads {2c, 2c+1} for all batches, i.e.
d-columns [128c, 128c+128) of q/k/v/out. Each core runs independent causal
attention for its 8 (batch, head) pairs; no collectives.

Per-core algorithm (per head):
  - load q/k (2 heads packed, [128, 1024] SBUF tiles), transpose to qT/kT
    [hd, S] layouts via PE matmuls against an identity (head pair stacked on
    partitions 0:64 / 64:128).
  - scoresT[kc, qr] = kT.T @ qT via PE, blocks [128kc x <=512qr], causally
    skipping fully-masked blocks; partial blocks only compute qr >= block
    start.  Blocks are packed into 2-bank PSUM tiles so one ACT exp covers
    up to 1024 columns.
  - exp(scale * scoresT) on ACT (no max subtraction: q,k ~ N(0,1) => scores
    are O(6), fp32 exp cannot overflow), output in pv_dt.  The causal
    triangle of each diagonal block is zeroed in-place by gpsimd
    affine_select (Pool engine is otherwise idle).
  - out[qr, hd] and the softmax denominator come from one PE accumulation:
    lhsT = expT block slices [kc, qr], rhs = v_aug [kc, 65] (v plus a ones
    column, prepared host-side), accumulated over kc chunks into PSUM.
  - normalize with per-row reciprocal of column 64, write packed out tile,
    DMA out.

Identity and the ones-augmented v are host-prepared ExternalInputs, which
keeps each instruction's producer set small (TRN2 instructions have one HW
semaphore-wait slot; split_multi_waits() legalizes the rest).
"""

import sys

sys.path.insert(0, "/opt/trn_rl_repo")

import numpy as np

import concourse.bass as bass
import concourse.mybir as mybir
import concourse.tile as tile
from concourse import bass_utils

B, S, D, H = 4, 1024, 1024, 16
HD = 64
NCORES = 8
HPC = H // NCORES          # heads per core = 2
CW = HPC * HD              # per-core d-column width = 128
P = 128                    # partitions
NT = S // P                # 8 s-tiles of 128
QCH = 512                  # qr chunk width
NQC = S // QCH             # 2 qr chunks per head
GROUP_BANKS = 2            # PSUM banks per exp group (one ACT exp per group)
SCALE = HD ** -0.5
FP32 = mybir.dt.float32

# score_dt: dtype of q/k on chip (PE QK^T matmul + transpose matmuls).
# pv_dt: dtype of exp(scores) and v (PE PV matmul).
# fp16 runs the PE at 1 cycle/row (same as bf16) with a 10-bit mantissa:
# |q|,|k| <= ~6 and exp(scores) <= ~500 are far inside fp16 range.
SCORE_DT = mybir.dt.float16
SCORE_NP = np.float16
PV_DT = mybir.dt.float16
PV_NP = np.float16


def split_multi_waits(nc):
    """TRN2 TPB instructions carry exactly one semaphore wait slot; walrus
    refuses >1 on_wait per instruction.  Hoist extra waits onto standalone
    EventSemaphore instructions on the same engine, inserted right before the
    owning instruction (engines dispatch in order, so semantics are
    unchanged)."""
    ctr = [0]
    for fn in nc.m.functions:
        for blk in fn.blocks:
            insts = list(blk.instructions)
            out = []
            changed = False
            for inst in insts:
                si = inst.sync_info
                if si is not None and len(si.on_wait) > 1:
                    changed = True
                    waits = list(si.on_wait)
                    for w in waits[:-1]:
                        ev = mybir.InstEventSemaphore(
                            name=f"evw-split-{ctr[0]}", ins=[], outs=[]
                        )
                        ctr[0] += 1
                        ev.engine = inst.engine
                        ev.sync_info = mybir.SyncInfo(on_wait=[w], on_update=[])
                        out.append(ev)
                    inst.sync_info = mybir.SyncInfo(
                        on_wait=[waits[-1]], on_update=list(si.on_update)
                    )
                out.append(inst)
            if changed:
                for i, inst in enumerate(out):
                    existing = blk.instructions
                    if i < len(existing) and existing[i].name == inst.name:
                        continue
                    blk.instructions.insert(i, inst)


def _head_blocks():
    """Block schedule for one head.

    Returns (groups, exp_cols) where groups is a list of psum groups; each
    group is a list of blocks (c, j, r, w, qs, goff, xoff):
      c: qr 512-chunk, j: kc 128-chunk, w: computed qr width,
      qs: global qr start, goff: column offset inside the 2-bank psum group,
      xoff: column offset of the block in the per-head expT tile.
    Blocks never cross a 512 (psum bank) boundary.
    """
    blocks = []
    for c in range(NQC):
        for j in range(4 * c + 4):
            r = max(0, j - 4 * c)
            w = QCH - P * r
            blocks.append((c, j, r, w, c * QCH + P * r))

    cap = GROUP_BANKS * 512
    membership = []
    cur = []
    goff = 0
    for (c, j, r, w, qs) in blocks:
        off = goff
        if off % 512 + w > 512:
            off = (off + 511) // 512 * 512
        if off + w > cap:
            membership.append(cur)
            cur = []
            off = 0
        cur.append((c, j, r, w, qs))
        goff = off + w
    if cur:
        membership.append(cur)

    # Lay blocks out within each group by first-fit-decreasing into 512-col
    # banks so the exp span never covers unused columns.
    groups = []
    xbase = 0
    for mem in membership:
        banks = []
        for blk in sorted(mem, key=lambda t: -t[3]):
            for bank in banks:
                if sum(x[3] for x in bank) + blk[3] <= 512:
                    bank.append(blk)
                    break
            else:
                banks.append([blk])
        cur = []
        span = 0
        for bi, bank in enumerate(banks):
            off = bi * 512
            for (c, j, r, w, qs) in bank:
                cur.append((c, j, r, w, qs, off, xbase + off))
                off += w
            span = max(span, off)
        cur.sort(key=lambda t: (t[0], t[1]))
        groups.append(cur)
        xbase += span
    return groups, xbase



def _pair_blocks():
    """Blocks for both heads of a batch, packed into shared 2-bank groups."""
    blocks = []
    for hl in range(HPC):
        for c in range(NQC):
            for j in range(4 * c + 4):
                r = max(0, j - 4 * c)
                w = QCH - P * r
                blocks.append((hl, c, j, r, w, c * QCH + P * r))
    cap = GROUP_BANKS * 512
    membership, cur, goff = [], [], 0
    for blk in blocks:
        off = goff
        if off % 512 + blk[4] > 512:
            off = (off + 511) // 512 * 512
        if off + blk[4] > cap:
            membership.append(cur); cur = []; off = 0
        cur.append(blk)
        goff = off + blk[4]
    if cur:
        membership.append(cur)
    groups, xbase = [], 0
    for mem in membership:
        banks = []
        for blk in sorted(mem, key=lambda t: -t[4]):
            for bank in banks:
                if sum(x[4] for x in bank) + blk[4] <= 512:
                    bank.append(blk); break
            else:
                banks.append([blk])
        cur, span = [], 0
        for bi, bank in enumerate(banks):
            off = bi * 512
            for (hl, c, j, r, w, qs) in bank:
                cur.append((hl, c, j, r, w, qs, off, xbase + off))
                off += w
            span = max(span, off)
        cur.sort(key=lambda t: (t[0], t[1], t[2]))
        groups.append(cur)
        xbase += span
    return groups, xbase


def build_program(repeat: int = 1, score_dt=SCORE_DT, pv_dt=PV_DT):
    nc = bass.Bass(trn_type="TRN2")
    qk_d = nc.dram_tensor("qk", [B, 2, S, CW], score_dt, kind="ExternalInput")
    va_d = nc.dram_tensor("value_aug", [B, HPC, S, HD + 1], pv_dt,
                          kind="ExternalInput")
    id_d = nc.dram_tensor("ident_in", [P, P], score_dt, kind="ExternalInput")
    o_d = nc.dram_tensor("attn_out", [B, S, CW], FP32, kind="ExternalOutput")

    groups, exp_cols = _pair_blocks()

    with tile.TileContext(nc) as tc:
        with (
            tc.tile_pool(name="const", bufs=1) as constp,
            tc.tile_pool(name="io", bufs=2) as iop,
            tc.tile_pool(name="outp", bufs=2) as outp,
            tc.tile_pool(name="trp", bufs=2) as trp,
            tc.tile_pool(name="vaugp", bufs=3) as vaugp,
            tc.tile_pool(name="expp", bufs=3) as expp,
            tc.tile_pool(name="smallp", bufs=8) as smallp,
            tc.tile_pool(name="ptr", bufs=2, space="PSUM") as ptr,
            tc.tile_pool(name="psc", bufs=2, space="PSUM") as psc,
            tc.tile_pool(name="pout", bufs=2, space="PSUM") as pout,
        ):
            ident = constp.tile([P, P], score_dt)
            nc.sync.dma_start(ident, id_d[:])

            for b_rep in range(repeat * B):
                b = b_rep % B
                # packed load: col x*1024 + t*128 + jj <-> dram[x, t*128+p, jj]
                qk_sb = iop.tile([P, 2 * S], score_dt, tag="qk_sb")
                if b_rep == 0:
                    # split the first load so the first transposes and score
                    # groups start ~2.5us earlier (shorter pipeline fill);
                    # later iterations are fully hidden and use one DMA.
                    for x in range(2):
                        for lo, hi in ((0, 512), (512, S)):
                            nc.sync.dma_start(
                                qk_sb[:, x * S + lo:x * S + hi].rearrange(
                                    "p (t j) -> p t j", j=CW),
                                qk_d[b, x, lo:hi].rearrange(
                                    "(t p) j -> p t j", p=P),
                            )
                else:
                    nc.sync.dma_start(
                        qk_sb.rearrange("p (x t j) -> p x t j", x=2, j=CW),
                        qk_d[b].rearrange("x (t p) j -> p x t j", p=P),
                    )
                q_sb = qk_sb[:, 0:S]
                k_sb = qk_sb[:, S:2 * S]

                # transpose to qT/kT [128, S]: partition p<64 -> head0 d=p,
                # p>=64 -> head1 d=p-64; column = s.  out = src_slice.T @ I.
                qT = trp.tile([P, S], score_dt, tag="qT")
                kT = trp.tile([P, S], score_dt, tag="kT")
                for src, dst in ((q_sb, qT), (k_sb, kT)):
                    for half in range(2):
                        pt_t = ptr.tile([P, 512], FP32, tag="pt")
                        for tt in range(4):
                            t = half * 4 + tt
                            nc.tensor.matmul(
                                pt_t[:, tt * P:(tt + 1) * P],
                                src[:, t * P:(t + 1) * P],
                                ident,
                                start=True,
                                stop=True,
                            )
                        nc.vector.tensor_copy(
                            dst[:, half * 512:(half + 1) * 512], pt_t[:]
                        )

                out_sb = outp.tile([P, S], FP32, tag="out_sb")

                # v with ones column, both heads: [128, 2*8*65].
                v_aug = vaugp.tile([P, HPC * NT * (HD + 1)], pv_dt, tag="v_aug")
                nc.sync.dma_start(
                    v_aug.rearrange("p (h j e) -> p h j e", h=HPC, e=HD + 1),
                    va_d[b].rearrange("h (j p) e -> p h j e", p=P),
                )

                v_views = [v_aug.rearrange(
                    "p (h j e) -> p h j e", h=HPC, e=HD + 1)[:, hl]
                    for hl in range(HPC)]
                expT = expp.tile([P, exp_cols], pv_dt, tag="expT")
                xoff_of = {}
                for grp in groups:
                    span = max(g[6] + g[4] for g in grp)
                    ps_t = psc.tile([P, GROUP_BANKS * 512], FP32, tag="ps")
                    for (hl, c, j, r, w, qs, goff, xoff) in grp:
                        xoff_of[(hl, c, j)] = xoff
                        pbase = hl * HD
                        nc.tensor.matmul(
                            ps_t[:, goff:goff + w],
                            kT[pbase:pbase + HD, j * P:(j + 1) * P],
                            qT[pbase:pbase + HD, qs:qs + w],
                            start=True, stop=True,
                        )
                    x0 = grp[0][7] - grp[0][6]
                    nc.scalar.activation(
                        expT[:, x0:x0 + span], ps_t[:, 0:span],
                        mybir.ActivationFunctionType.Exp, scale=SCALE,
                    )
                    for (hl, c, j, r, w, qs, goff, xoff) in grp:
                        if j >= 4 * c:
                            nc.gpsimd.affine_select(
                                out=expT[:, xoff:xoff + P],
                                in_=expT[:, xoff:xoff + P],
                                compare_op=mybir.AluOpType.is_ge,
                                fill=0.0, base=0, pattern=[[1, P]],
                                channel_multiplier=-1,
                            )
                for hl in range(HPC):
                    v_view = v_views[hl]
                    for c in range(NQC):
                        po4 = pout.tile([P, 4 * (HD + 1)], FP32, tag="po")
                        for qi in range(4):
                            qt = c * 4 + qi
                            for j in range(qt + 1):
                                r = max(0, j - 4 * c)
                                qs = c * QCH + P * r
                                o0 = xoff_of[(hl, c, j)] + qt * P - qs
                                nc.tensor.matmul(
                                    po4[:, qi * (HD + 1):(qi + 1) * (HD + 1)],
                                    expT[:, o0:o0 + P],
                                    v_view[:, j, :],
                                    start=(j == 0), stop=(j == qt),
                                )
                        po_v = po4.rearrange("p (t e) -> p t e", e=HD + 1)
                        recip4 = smallp.tile([P, 4], FP32, tag="recip")
                        rv = recip4.rearrange("p (t o) -> p t o", o=1)
                        nc.vector.reciprocal(rv, po_v[:, :, HD:HD + 1])
                        out_v = out_sb.rearrange("p (t j) -> p t j", j=P)[
                            :, c * 4:(c + 1) * 4, hl * HD:(hl + 1) * HD]
                        nc.vector.tensor_mul(
                            out_v, po_v[:, :, 0:HD], rv.broadcast_to((P, 4, HD)))
                nc.sync.dma_start(
                    o_d[b].rearrange("(t p) j -> p t j", p=P),
                    out_sb.rearrange("p (t j) -> p t j", j=CW),
                )
    split_multi_waits(nc)
    return nc


_PROGRAM = None

_IDENT = np.eye(P, dtype=SCORE_NP)


def make_in_maps(query, key, value, pv_np=PV_NP):
    query = np.asarray(query, dtype=np.float32)
    key = np.asarray(key, dtype=np.float32)
    value = np.asarray(value, dtype=np.float32)
    in_maps = []
    for c in range(NCORES):
        sl = slice(c * CW, (c + 1) * CW)
        v_shard = value[:, :, sl]
        v_aug = np.ones((B, HPC, S, HD + 1), dtype=pv_np)
        for hl in range(HPC):
            v_aug[:, hl, :, :HD] = v_shard[:, :, hl * HD:(hl + 1) * HD].astype(pv_np)
        qk = np.stack([query[:, :, sl], key[:, :, sl]], axis=1).astype(SCORE_NP)
        in_maps.append(
            {
                "qk": np.ascontiguousarray(qk),
                "value_aug": v_aug,
                "ident_in": _IDENT,
            }
        )
    return in_maps


_RUNNER = None


def _get_runner():
    """Build the Bass program once and return a cached jitted 8-core runner
    (mirrors bass2jax.run_bass_via_pjrt's shard_map path; re-invoking
    run_bass_kernel_spmd would re-trace and re-jit on every call)."""
    global _RUNNER
    if _RUNNER is not None:
        return _RUNNER
    import jax
    from jax.sharding import Mesh, PartitionSpec
    from jax.experimental.shard_map import shard_map
    from concourse import bass2jax

    nc = build_program()
    bass2jax.install_neuronx_cc_hook()

    partition_name = nc.partition_id_tensor.name if nc.partition_id_tensor else None
    in_names, out_names, out_avals, zero_outs = [], [], [], []
    for alloc in nc.m.functions[0].allocations:
        if not isinstance(alloc, mybir.MemoryLocationSet):
            continue
        name = alloc.memorylocations[0].name
        if alloc.kind == "ExternalInput":
            if name != partition_name:
                in_names.append(name)
        elif alloc.kind == "ExternalOutput":
            shape = tuple(alloc.tensor_shape)
            dtype = mybir.dt.np(alloc.dtype)
            out_names.append(name)
            out_avals.append(jax.core.ShapedArray(shape, dtype))
            zero_outs.append(np.zeros(shape, dtype))
    n_params = len(in_names)
    all_in_names = list(in_names) + list(out_names)
    if partition_name is not None:
        all_in_names.append(partition_name)

    def _body(*args):
        operands = list(args)
        if partition_name is not None:
            operands.append(bass2jax.partition_id_tensor())
        outs = bass2jax._bass_exec_p.bind(
            *operands,
            out_avals=tuple(out_avals),
            in_names=tuple(all_in_names),
            out_names=tuple(out_names),
            lowering_input_output_aliases=(),
            sim_require_finite=True,
            sim_require_nnan=True,
            nc=nc,
        )
        return tuple(outs)

    devices = jax.devices()[:NCORES]
    mesh = Mesh(np.asarray(devices), ("core",))
    spec = PartitionSpec("core")
    fn = jax.jit(
        shard_map(_body, mesh=mesh,
                  in_specs=(spec,) * (n_params + len(out_names)),
                  out_specs=(spec,) * len(out_names), check_rep=False),
        keep_unused=True,
    )
    _RUNNER = (fn, in_names, out_names, out_avals, zero_outs)
    return _RUNNER


def _concat_inputs(query, key, value):
    """Vectorized equivalent of concatenating make_in_maps() over cores:
    returns {name: [(8*dim0), ...] array} keyed like the ExternalInputs."""
    q16 = np.asarray(query, dtype=SCORE_NP).reshape(B, S, NCORES, CW)
    k16 = np.asarray(key, dtype=SCORE_NP).reshape(B, S, NCORES, CW)
    # qk: per core [B, 2, S, CW] -> concat [(8B), 2, S, CW]
    qk = np.stack(
        [q16.transpose(2, 0, 1, 3), k16.transpose(2, 0, 1, 3)], axis=2
    ).reshape(NCORES * B, 2, S, CW)
    v16 = np.asarray(value, dtype=PV_NP).reshape(B, S, NCORES, HPC, HD)
    v_aug = np.ones((NCORES, B, HPC, S, HD + 1), dtype=PV_NP)
    v_aug[..., :HD] = v16.transpose(2, 0, 3, 1, 4)
    v_aug = v_aug.reshape(NCORES * B, HPC, S, HD + 1)
    ident = np.broadcast_to(_IDENT, (NCORES, P, P)).reshape(NCORES * P, P)
    return {
        "qk": np.ascontiguousarray(qk),
        "value_aug": np.ascontiguousarray(v_aug),
        "ident_in": np.ascontiguousarray(ident),
    }


def kernel(query: np.ndarray, key: np.ndarray, value: np.ndarray) -> np.ndarray:
    fn, in_names, out_names, out_avals, zero_outs = _get_runner()
    cat = _concat_inputs(query, key, value)
    concat_in = [cat[name] for name in in_names]
    concat_zeros = [
        np.zeros((NCORES * z.shape[0], *z.shape[1:]), z.dtype) for z in zero_outs
    ]
    out_arrs = fn(*concat_in, *concat_zeros)
    oi = out_names.index("attn_out")
    full = np.asarray(out_arrs[oi]).reshape(NCORES, *out_avals[oi].shape)
    return np.concatenate(list(full), axis=2)

